# revision 29
# baseline (speedup 1.0000x reference)
"""Multi-head attention (B=1, S=4096, D=1024, H=16, causal) on 8 Trainium2
NeuronCores.

Sharding: tensor-parallel over heads — each core owns 2 heads (128 of the
1024 projection dims). Wq/Wk/Wv are split column-wise, Wo row-wise; each
core computes a full [S, D] partial of the output projection (bf16) and the
all-reduce is done on the host by summing the 8 partials (+ Wo_b once).

All matmul operands are bf16 (f32 PSUM accumulation): same 1 cycle/row PE
rate as f32r but FWL-eligible weight loads, half the DMA/SBUF traffic, and
2x DVE modes where applicable.

Per-core device kernel:
  qT/kT projections produce [c=128, S] bf16 directly (contract D streams
  from host-pretransposed inputs); the two heads live on partition halves
  0-63 / 64-127 so the per-head score matmuls (contract 64) auto-derive
  PE row tiles (0,0)/(64,0) and run concurrently in the array.
  v is projected directly into [s, c] layout (x-subtile stationary) and
  bias-added into an augmented [s, 65]-per-head slot (ones column => softmax
  denominator falls out of the attn@V matmul as PSUM row 64).
  Scores are computed transposed (scoresT[k, q]) so softmax exp is the PSUM
  eviction (ACT, scale=1/8, bf16 out); the partial diagonal 128-bands are
  zeroed by a Pool-engine mask multiply; fully-masked blocks are skipped.
  The v slots carry a 64-wide ones block, so the attn@V matmul lands the
  softmax denominator replicated on PSUM partitions 0-63 (numerators on
  64-127): normalization is reciprocal_approx_fast + one multiply, no
  broadcast needed. The normalized bf16 [c, q] tiles for both heads land in
  one [128, q] tile so the final Wo projection is a single K=128 matmul per
  output block. Projections and Wo blocks are emitted as filler units
  interleaved into the attention pair loop, keeping both the PE dense (HAM
  stays un-throttled) and the ACT exp stream gapless across s-tiles;
  attn@V lags its exp by one pair so the in-order PE queue never stalls.
"""

import numpy as np
import ml_dtypes

D = 1024
H = 16
DK = D // H  # 64
S = 4096
NCORES = 8
CD = 128          # c-dims (2 heads) per core
ST = 512          # s/q tile
NST = S // ST     # 8
KB = 128          # k block
NKB = S // KB     # 32
SLOT = 128        # v_sb cols per head per k-block (64 dims + 64 ones)

_compiled = [None]


def _build():
    import concourse.bacc as bacc
    import concourse.mybir as mybir
    import concourse.tile as tile

    f32 = mybir.dt.float32
    f32r = mybir.dt.float32r
    bf16 = mybir.dt.bfloat16
    EXP = mybir.ActivationFunctionType.Exp
    MULT = mybir.AluOpType.mult
    ADD = mybir.AluOpType.add

    nc = bacc.Bacc(None, target_bir_lowering=False)

    XQ = nc.dram_tensor("xq", [128, 8, S], bf16, kind="ExternalInput")
    XK = nc.dram_tensor("xk", [128, 8, S], bf16, kind="ExternalInput")
    XV = nc.dram_tensor("xv", [128, 8, S], bf16, kind="ExternalInput")
    WQ = nc.dram_tensor("wq", [128, 8, CD], bf16, kind="ExternalInput")
    WK = nc.dram_tensor("wk", [128, 8, CD], bf16, kind="ExternalInput")
    WV = nc.dram_tensor("wv", [128, 8, CD], bf16, kind="ExternalInput")
    BQ = nc.dram_tensor("bq", [CD, 1], f32, kind="ExternalInput")
    BK = nc.dram_tensor("bk", [CD, 1], f32, kind="ExternalInput")
    BVB = nc.dram_tensor("bvb", [128, 512], bf16, kind="ExternalInput")
    WOR = nc.dram_tensor("wor", [CD, D], bf16, kind="ExternalInput")
    MSK = nc.dram_tensor("msk", [KB, KB], bf16, kind="ExternalInput")
    OUT = nc.dram_tensor("out", [S, D], bf16, kind="ExternalOutput")

    with tile.TileContext(nc) as tc:
        with (
            tc.tile_pool(name="const", bufs=1) as const,
            tc.tile_pool(name="qin", bufs=3) as qin_p,
            tc.tile_pool(name="kin", bufs=3) as kin_p,
            tc.tile_pool(name="vin", bufs=3) as vin_p,
            tc.tile_pool(name="expp", bufs=6) as exp_p,
            tc.tile_pool(name="rsb", bufs=4) as rsb_p,
            tc.tile_pool(name="wlp", bufs=3) as wl_p,
            tc.tile_pool(name="oout", bufs=6) as oout_p,
            tc.tile_pool(name="psA", bufs=2, space="PSUM") as psA,
            tc.tile_pool(name="psS", bufs=2, space="PSUM") as psS,
            tc.tile_pool(name="psO", bufs=2, space="PSUM") as psO,
        ):
            # ---- static SBUF tensors ----
            qT_sb = const.tile([CD, S], bf16, tag="qTl")
            kT_sb = const.tile([CD, S], bf16, tag="kT")
            v_sb = const.tile([128, NKB, 2 * SLOT], bf16, tag="vsb")

            wq_sb = const.tile([128, 8, CD], bf16, tag="wq")
            wk_sb = const.tile([128, 8, CD], bf16, tag="wk")
            wv_sb = const.tile([128, 8, CD], bf16, tag="wv")
            woR = const.tile([CD, D], bf16, tag="woR")
            mask_sb = const.tile([KB, KB], bf16, tag="mask")
            bq_sb = const.tile([CD, 1], f32, tag="bq")
            bk_sb = const.tile([CD, 1], f32, tag="bk")
            bvb_sb = const.tile([128, 512], bf16, tag="bvb")

            woL_tiles = {}
            prefetched = {}

            def fetch(st, src, in_pool, name):
                xin = in_pool.tile([128, 8, ST], bf16, tag="xin",
                                   name=f"xin_{name}{st}")
                for g in range(2):
                    nc.sync.dma_start(
                        out=xin[:, 4 * g : 4 * g + 4, :],
                        in_=src[:, 4 * g : 4 * g + 4,
                                st * ST : (st + 1) * ST],
                    )
                return xin

            # critical consts first (first proj matmuls need these);
            # first halves of the weights land before the second halves so
            # the t=0..3 projection matmuls can start as early as possible
            for g in range(2):
                nc.sync.dma_start(out=wq_sb[:, 4 * g : 4 * g + 4, :],
                                  in_=WQ[:, 4 * g : 4 * g + 4, :])
                nc.sync.dma_start(out=wk_sb[:, 4 * g : 4 * g + 4, :],
                                  in_=WK[:, 4 * g : 4 * g + 4, :])
            nc.sync.dma_start(out=wv_sb[:], in_=WV[:])
            nc.sync.dma_start(out=bq_sb[:], in_=BQ[:])
            nc.sync.dma_start(out=bk_sb[:], in_=BK[:])
            nc.sync.dma_start(out=bvb_sb[:], in_=BVB[:])
            prefetched[("q", 0)] = fetch(0, XQ, qin_p, "q")
            prefetched[("k", 0)] = fetch(0, XK, kin_p, "k")
            prefetched[("v", 0)] = fetch(0, XV, vin_p, "v")

            # bulky / later-needed consts
            nc.sync.dma_start(out=mask_sb[:], in_=MSK[:])
            nc.sync.dma_start(out=woR[:], in_=WOR[:])

            # ones blocks of the augmented v slots (cols 0-63 per head
            # slot => attn@V lands denominators on PSUM partitions 0-63,
            # numerators on 64-127)
            nc.gpsimd.memset(v_sb[:, :, 0:DK], 1.0)
            nc.gpsimd.memset(v_sb[:, :, SLOT : SLOT + DK], 1.0)

            def get_in(st, name, src, in_pool):
                xin = prefetched.pop((name, st), None)
                if xin is None:
                    xin = fetch(st, src, in_pool, name)
                return xin

            v4 = v_sb.rearrange("p n (h c) -> p n h c", h=2)
            bvb4 = bvb_sb.rearrange("p (k h c) -> p k h c", k=4, h=2)

            def proj_units(st, xq, xk, xv):
                """Projection of s-tile st as schedulable PE work units."""
                state = {}

                def qk_part(xin, w_sb, b_sb, dst_ap, key, lo, hi):
                    def run():
                        if key not in state:
                            state[key] = psA.tile([128, ST], f32, tag="pp",
                                                  name=f"pp{key}{st}")
                        ps = state[key]
                        for t in range(lo, hi):
                            nc.tensor.matmul(
                                ps[:],
                                lhsT=w_sb[:, t, :],
                                rhs=xin[:, t, :],
                                start=(t == 0),
                                stop=(t == 7),
                            )
                        if hi == 8:
                            nc.vector.tensor_scalar_add(dst_ap, ps[:],
                                                        b_sb[:])
                    return run

                def v_part(qb):
                    def run():
                        if "v" not in state:
                            state["v"] = psA.tile([128, 4, 128], f32,
                                                  tag="pp", name=f"pv{st}")
                        pv = state["v"]
                        for t in range(8):
                            nc.tensor.matmul(
                                pv[:, qb, :],
                                lhsT=xv[:, t, qb * 128 : (qb + 1) * 128],
                                rhs=wv_sb[:, t, :],
                                start=(t == 0),
                                stop=(t == 7),
                            )
                        if qb == 3:
                            # bias-add + pack into augmented slots (skip the
                            # ones columns); DVE: GPSIMD cannot read PSUM
                            nc.vector.tensor_tensor(
                                out=v4[:, 4 * st : 4 * st + 4, :, DK:SLOT],
                                in0=pv.rearrange("p k (h c) -> p k h c", h=2),
                                in1=bvb4[:],
                                op=ADD,
                            )
                    return run

                qdst = qT_sb[:, st * ST : (st + 1) * ST]
                kdst = kT_sb[:, st * ST : (st + 1) * ST]
                return [
                    qk_part(xq, wq_sb, bq_sb, qdst, "q", 0, 4),
                    qk_part(xq, wq_sb, bq_sb, qdst, "q", 4, 8),
                    qk_part(xk, wk_sb, bk_sb, kdst, "k", 0, 4),
                    qk_part(xk, wk_sb, bk_sb, kdst, "k", 4, 8),
                    v_part(0), v_part(1), v_part(2), v_part(3),
                ]

            def attn(qt, filler):
                npr = 2 * qt + 2
                po = {}
                for h in (0, 1):
                    po[h] = psO.tile([128, ST], f32, tag="po",
                                     name=f"po{qt}_{h}")

                def attnv(pr, ex):
                    # attn @ V (+ones col => denominators in PSUM row 64)
                    for h in (0, 1):
                        for j in range(2):
                            kb = 2 * pr + j
                            rel = kb - 4 * qt
                            c0 = 128 * rel if rel > 0 else 0
                            nc.tensor.matmul(
                                po[h][:, c0:ST],
                                lhsT=v_sb[:, kb, h * SLOT : (h + 1) * SLOT],
                                rhs=ex[h][:, j * ST + c0 : (j + 1) * ST],
                                start=(pr == 0 and j == 0),
                                stop=(pr == npr - 1 and j == 1),
                            )

                prev = None  # (pr, ex) whose attn@V is still pending
                for pr in range(npr):
                    rels = [2 * pr + j - 4 * qt for j in (0, 1)]
                    ps = {}
                    for h in (0, 1):
                        ps[h] = psS.tile([128, 2 * ST], f32, tag="ps",
                                         name=f"ps{qt}_{h}_{pr}")
                    # scores: head-interleaved so the two K=64 matmuls run
                    # in different PE row-groups concurrently
                    for j in range(2):
                        kb = 2 * pr + j
                        rel = rels[j]
                        c0 = 128 * rel if rel > 0 else 0
                        for h in (0, 1):
                            nc.tensor.matmul(
                                ps[h][:, j * ST + c0 : (j + 1) * ST],
                                lhsT=kT_sb[64 * h : 64 * h + 64,
                                           kb * KB : (kb + 1) * KB],
                                rhs=qT_sb[64 * h : 64 * h + 64,
                                          qt * ST + c0 : (qt + 1) * ST],
                                start=True,
                                stop=True,
                            )
                    ex = {}
                    for h in (0, 1):
                        ex[h] = exp_p.tile([128, 2 * ST], bf16, tag="ex",
                                           name=f"ex{qt}_{h}_{pr}")
                        if rels[0] >= 2:  # steep diagonal pair: narrow exps
                            for j in range(2):
                                c0 = 128 * rels[j]
                                nc.scalar.activation(
                                    ex[h][:, j * ST + c0 : (j + 1) * ST],
                                    ps[h][:, j * ST + c0 : (j + 1) * ST],
                                    EXP, scale=0.125,
                                )
                        else:
                            nc.scalar.activation(ex[h][:], ps[h][:], EXP,
                                                 scale=0.125)
                    # zero the partial diagonal 128-bands (Pool engine)
                    for h in (0, 1):
                        for j in range(2):
                            rel = rels[j]
                            if rel >= 0:
                                b0 = j * ST + 128 * rel
                                nc.gpsimd.tensor_tensor(
                                    out=ex[h][:, b0 : b0 + 128],
                                    in0=ex[h][:, b0 : b0 + 128],
                                    in1=mask_sb[:],
                                    op=MULT,
                                )
                    # attn@V lags one pair so PE never stalls on this exp
                    if prev is not None:
                        attnv(*prev)
                    prev = (pr, ex)
                    # interleave pending proj/Wo units, spread evenly
                    filler(-(npr - pr))
                attnv(*prev)
                # normalize: woL[h*64:(h+1)*64, :] = po[h][0:64] / denom
                woL = wl_p.tile([128, ST], bf16, tag="wl", name=f"wl{qt}")
                for h in (0, 1):
                    r_sb = rsb_p.tile([DK, ST], f32, tag="r",
                                      name=f"r{qt}_{h}")
                    nc.vector.reciprocal_approx_fast(out=r_sb[:],
                                                     in_=po[h][0:64, :])
                    nc.vector.tensor_tensor(
                        out=woL[64 * h : 64 * h + 64, :],
                        in0=po[h][64:128, :], in1=r_sb[:], op=MULT,
                    )
                woL_tiles[qt] = woL

            def wo_units(qt):
                """8 closures, each one output block of the Wo projection."""
                wl = woL_tiles.pop(qt)

                def unit(qb, nt):
                    def run():
                        q0 = qt * ST + qb * 128
                        pw = psA.tile([128, ST], f32, tag="pp",
                                      name=f"pw{qt}_{qb}_{nt}")
                        nc.tensor.matmul(
                            pw[:],
                            lhsT=wl[:, qb * 128 : (qb + 1) * 128],
                            rhs=woR[:, nt * ST : (nt + 1) * ST],
                            start=True, stop=True,
                        )
                        ob = oout_p.tile([128, ST], bf16, tag="ob",
                                         name=f"ob{qt}_{qb}_{nt}")
                        nc.vector.tensor_copy(ob[:], pw[:])
                        nc.sync.dma_start(
                            out=OUT[q0 : q0 + 128, nt * ST : (nt + 1) * ST],
                            in_=ob[:],
                        )
                    return run

                return [unit(qb, nt) for qb in range(4) for nt in range(2)]

            pending = []

            reserve = [4]

            def filler(n):
                # n < 0: spread -> emit ceil((len-R)/|n|), holding ~R units
                # back as PE filler for the next tile's ACT-bound start;
                # n > 0: emit up to n (force-drain)
                if n < 0:
                    n = -((len(pending) - reserve[0]) // n)
                for _ in range(min(n, len(pending))):
                    pending.pop(0)()

            for st in range(NST):
                if st == 0:
                    # first projection runs inline (nothing to overlap yet)
                    xq = get_in(0, "q", XQ, qin_p)
                    xk = get_in(0, "k", XK, kin_p)
                    xv = get_in(0, "v", XV, vin_p)
                    for u in proj_units(0, xq, xk, xv):
                        u()
                # leftover proj units for this tile must precede its scores
                filler(len(pending))
                if st + 1 < NST:
                    xq = fetch(st + 1, XQ, qin_p, "q")
                    xk = fetch(st + 1, XK, kin_p, "k")
                    xv = fetch(st + 1, XV, vin_p, "v")
                    pending.extend(proj_units(st + 1, xq, xk, xv))
                if st >= 1:
                    # wo for qt=st-1: drained by filler inside attn(st)
                    pending.extend(wo_units(st - 1))
                if st == NST - 1:
                    reserve[0] = 0  # nothing follows: drain fully
                attn(st, filler)
            # drain the tail
            filler(len(pending))
            pending.extend(wo_units(NST - 1))
            filler(len(pending))

    nc.compile()
    return nc


def _prep_inputs(Q, K, V, Wq_w, Wq_b, Wk_w, Wk_b, Wv_w, Wv_b, Wo_w, Wo_b):
    bf = ml_dtypes.bfloat16
    f = np.float32

    def xprep(X):
        # [S, D] -> [128, 8, S]: x[p, t, s] = X[s, t*128+p]
        return np.ascontiguousarray(
            X[0].T.reshape(8, 128, S).transpose(1, 0, 2).astype(bf)
        )

    def wprep(Wslice):
        # Wslice [CD, D] -> [128, 8, CD]: w[p, t, c] = Wslice[c, t*128+p]
        return np.ascontiguousarray(
            Wslice.T.reshape(8, 128, CD).transpose(1, 0, 2).astype(bf)
        )

    XQp, XKp, XVp = xprep(Q), xprep(K), xprep(V)
    p = np.arange(KB)[:, None]
    fidx = np.arange(KB)[None, :]
    msk = np.where(p <= fidx, 1.0, 0.0).astype(bf)
    WoT = np.ascontiguousarray(Wo_w.T, dtype=f)  # [in, out]

    in_maps = []
    for c in range(NCORES):
        c0 = CD * c
        in_maps.append({
            "xq": XQp, "xk": XKp, "xv": XVp,
            "wq": wprep(Wq_w[c0 : c0 + CD, :]),
            "wk": wprep(Wk_w[c0 : c0 + CD, :]),
            "wv": wprep(Wv_w[c0 : c0 + CD, :]),
            "bq": np.ascontiguousarray(Wq_b[c0 : c0 + CD, None], dtype=f),
            "bk": np.ascontiguousarray(Wk_b[c0 : c0 + CD, None], dtype=f),
            "bvb": np.ascontiguousarray(
                np.broadcast_to(np.tile(Wv_b[c0 : c0 + CD], 4), (128, 512))
            ).astype(bf),
            "wor": np.ascontiguousarray(WoT[c0 : c0 + CD, :], dtype=bf),
            "msk": msk,
        })
    return in_maps


def _numpy_fallback(Q, K, V, Wq_w, Wq_b, Wk_w, Wk_b, Wv_w, Wv_b, Wo_w, Wo_b,
                    mask):
    q = (Q @ Wq_w.T + Wq_b).reshape(1, S, H, DK).transpose(0, 2, 1, 3)
    k = (K @ Wk_w.T + Wk_b).reshape(1, S, H, DK).transpose(0, 2, 1, 3)
    v = (V @ Wv_w.T + Wv_b).reshape(1, S, H, DK).transpose(0, 2, 1, 3)
    scores = np.einsum("bhqd,bhkd->bhqk", q, k) / np.sqrt(DK).astype(np.float32)
    scores = np.where(mask == 0, np.float32(-1e9), scores)
    scores -= scores.max(axis=-1, keepdims=True)
    e = np.exp(scores)
    attn = e / e.sum(axis=-1, keepdims=True)
    out = np.einsum("bhqk,bhkd->bhqd", attn, v)
    out = out.transpose(0, 2, 1, 3).reshape(1, S, D)
    return (out @ Wo_w.T + Wo_b).astype(np.float32)


def kernel(Q, K, V, Wq_w, Wq_b, Wk_w, Wk_b, Wv_w, Wv_b, Wo_w, Wo_b, mask,
           **run_kwargs):
    Q = np.asarray(Q); K = np.asarray(K); V = np.asarray(V)
    Wq_w = np.asarray(Wq_w); Wq_b = np.asarray(Wq_b)
    Wk_w = np.asarray(Wk_w); Wk_b = np.asarray(Wk_b)
    Wv_w = np.asarray(Wv_w); Wv_b = np.asarray(Wv_b)
    Wo_w = np.asarray(Wo_w); Wo_b = np.asarray(Wo_b)
    mask = np.asarray(mask)

    causal = np.array_equal(
        mask.reshape(S, S), np.tril(np.ones((S, S), mask.dtype))
    )
    if not causal:
        return _numpy_fallback(Q, K, V, Wq_w, Wq_b, Wk_w, Wk_b, Wv_w, Wv_b,
                               Wo_w, Wo_b, mask)

    from concourse.bass_utils import run_bass_kernel_spmd

    if _compiled[0] is None:
        _compiled[0] = _build()
    nc = _compiled[0]

    in_maps = _prep_inputs(Q, K, V, Wq_w, Wq_b, Wk_w, Wk_b, Wv_w, Wv_b,
                           Wo_w, Wo_b)
    res = run_bass_kernel_spmd(nc, in_maps, list(range(NCORES)), **run_kwargs)
    out = np.zeros((S, D), np.float32)
    for cres in res.results:
        out += np.asarray(cres["out"], dtype=np.float32)
    out += Wo_b.astype(np.float32)
    if run_kwargs:
        kernel.last_result = res
    return out.reshape(1, S, D).astype(np.float32)
